# revision 15
# baseline (speedup 1.0000x reference)
"""FFM layer kernel for Trainium2, data-parallel over batch on 8 NeuronCores.

The reference computes, for each sample b:
    x = [dense(13) | onehot(26 fields x 1000)]            # [B, 26013]
    linear = w0 + x @ w                                   # [B, 1]
    field_f = einsum('bf,fik->bik', x, v)                 # [B, 39, 8]
    inter = 0.5*((sum_i field_f)^2.sum(k) - (field_f^2).sum(i,k))
    out = linear + inter

Because x is one-hot in the sparse block, x @ [v|w] is a 26-row gather from
an fp16 [26013, 384] table (cols 0..311 = flattened v row, col 312 = w,
313.. zero pad so each row is 768 B, a multiple of 256) plus a tiny fp32
dense [14]x[14,313] matmul (row 13 = ones row carrying w0 into col 312).
Each core handles 512 samples as 4 tiles of 128.  Each tile's 26
rows/sample are fetched by dma_gather calls; tile 0 uses FIVE chunks
(1/6/7/8/4 fields, the 1-field first chunk unblocks the gpsimd engine's
4-deep exec queue in ~1 us instead of ~7.5 so all four SWDGE queues
start generating descriptors immediately), tiles 1-3 use FOUR chunks
(7/7/8/4).  The queue assignment is hand-balanced so each of the 4
queues carries exactly 3328 rows.  The num_idxs count registers are
loaded once up front and the mlp library load is issued as the first
gpsimd instruction (the ~9 us Q7 library fetch is the hard lower bound
on the first gather).  The idx table is split-loaded on the two HWDGE
queues (sync + scalar).  The 26-row sum runs on the vector engine as a
minimal 10-add tree working in-place in the gather buffers, arranged so
everything except the last 4-field chunk is collapsed before that chunk
lands.  h1 = sum(f^2) plus the final (h2-h1)*0.5 + linear combine run on
the otherwise-idle activation engine in parallel with the DVE's s8
reduction; the [512,1] result is transposed with 4 DVE 32x32 stream
transposes so the final store is 4 contiguous 512 B rows.  Raw bacc
with manual semaphores -- no TileContext.
"""

import numpy as np

N_DENSE = 13
N_SPARSE = 26
ONEHOT = 1000
FIELD = 39
K = 8
FEAT = N_DENSE + N_SPARSE * ONEHOT  # 26013
B = 4096
NCORES = 8
BC = B // NCORES  # 512 samples per core
P = 128
NT = BC // P  # 4 tiles per core
D = FIELD * K  # 312
DW = D + 1  # 313 (col 312 carries the linear weight)
E = 384  # gathered fp16 row width (768 B, multiple of 256)
NI = N_SPARSE * P  # 3328 gathered rows per tile
TC = NI // 16  # 208 idx columns per tile
# per-tile sub-gathers: (first field, num fields).  7 small chunks per tile
# round-robined over the 4 SWDGE queues: 28 chunks x {512,384} rows gives
# every queue exactly 3328 rows AND keeps each queue's backlog within one
# chunk of the others, so the chunks a tile's first add needs (0-3, spread
# over all four queues) land in parallel at the per-queue desc-gen rate.
CHUNKS7 = ((0, 4), (4, 3), (7, 4), (11, 3), (14, 4), (18, 4), (22, 4))
TCHUNKS = (CHUNKS7,) * NT
TQUEUES = tuple(
    tuple((t * len(CHUNKS7) + c) % 4 for c in range(len(CHUNKS7)))
    for t in range(NT)
)
# vv increments per tile: tiles 0-2 leave rr/out to the act engine (13 ops);
# tile 3 keeps them on the DVE (15 ops) so the act chain is off the tail
TOPS = (13, 13, 13, 15)
TBASE = (0, 13, 26, 39)
AOPS = 3  # act-engine av increments per tile 0-2 (last op incs ah instead)

_cached_nc = None


def _build_program():
    global _cached_nc
    if _cached_nc is not None:
        return _cached_nc

    import concourse.bacc as bacc
    import concourse.mybir as mybir
    from concourse import library_config

    nc = bacc.Bacc(
        "TRN2",
        debug=False,
        enable_asserts=False,
        target_bir_lowering=False,
        num_devices=NCORES,
        num_swdge_queues=4,
        dynamic_dma_scratch_size=65536,
    )
    f32 = mybir.dt.float32
    f16 = mybir.dt.float16
    i16 = mybir.dt.int16
    add_op = mybir.AluOpType.add
    mult_op = mybir.AluOpType.mult
    act_fn = mybir.ActivationFunctionType

    table = nc.dram_tensor("table", [FEAT, E], f16, kind="ExternalInput").ap()
    idx = nc.dram_tensor("idx", [P, NT * TC], i16, kind="ExternalInput").ap()
    dnt = nc.dram_tensor("dnt", [N_DENSE + 1, BC], f32, kind="ExternalInput").ap()
    vdx = nc.dram_tensor("vdx", [N_DENSE + 1, DW], f32, kind="ExternalInput").ap()
    out = nc.dram_tensor("out", [BC, 1], f32, kind="ExternalOutput").ap()
    outT = out.rearrange("(t p) o -> t (p o)", t=NT)

    idx_sb = nc.alloc_sbuf_tensor("idx_sb", [P, NT * TC], i16).ap()
    dnt_sb = nc.alloc_sbuf_tensor("dnt_sb", [N_DENSE + 1, BC], f32).ap()
    vdx_sb = nc.alloc_sbuf_tensor("vdx_sb", [N_DENSE + 1, DW], f32).ap()
    g_sb = [nc.alloc_sbuf_tensor(f"g{t}", [P, 26 * E], f16).ap() for t in range(NT)]
    tot_sb = [nc.alloc_sbuf_tensor(f"tot{t}", [P, DW], f32).ap() for t in range(NT)]
    s8_sb = [nc.alloc_sbuf_tensor(f"s8_{t}", [P, K], f32).ap() for t in range(NT)]
    sqa_sb = nc.alloc_sbuf_tensor("sqa", [P, D], f16).ap()  # act-engine dump
    sq8_sb = [nc.alloc_sbuf_tensor(f"sq8_{s}", [P, K], f32).ap() for s in range(2)]
    h1_sb = [nc.alloc_sbuf_tensor(f"h1_{t}", [P, 1], f32).ap() for t in range(NT)]
    h2_sb = [nc.alloc_sbuf_tensor(f"h2_{t}", [P, 1], f32).ap() for t in range(NT)]
    rr_sb = [nc.alloc_sbuf_tensor(f"rr_{t}", [P, 1], f32).ap() for t in range(NT)]
    # [128, 32] fp32: col t holds tile t's result; cols 4..31 zeroed once so
    # the 32x32 stream transposes never read uninitialized SBUF
    ot4_sb = nc.alloc_sbuf_tensor("ot4", [P, 32], f32).ap()
    otT_sb = nc.alloc_sbuf_tensor("otT", [32, P], f32).ap()
    ps_ps = [nc.alloc_psum_tensor(f"ps{t}", [P, DW], f32).ap() for t in range(NT)]

    io_ixa = nc.alloc_semaphore("io_ixa")  # idx partitions 0..63 (sync q)
    io_ixb = nc.alloc_semaphore("io_ixb")  # idx partitions 64..127 (scalar q)
    io_dv = nc.alloc_semaphore("io_dv")    # dnt+vdx loads x 16 each
    st = nc.alloc_semaphore("st")          # output store x 16
    # one sem per sub-gather: a DMA sem may only be updated from one SWDGE
    # queue, and completions of two gathers on one queue interleave
    gs = [
        [nc.alloc_semaphore(f"gs{t}_{c}") for c in range(len(TCHUNKS[t]))]
        for t in range(NT)
    ]
    mm = nc.alloc_semaphore("mm")      # dense matmul done (per tile)
    ah = nc.alloc_semaphore("ah")      # act-engine tile output done
    av = nc.alloc_semaphore("av")      # act-engine same-engine RAW ordering
    dn2 = nc.alloc_semaphore("dn2")    # transposed result ready for store
    vv = nc.alloc_semaphore("vv")      # vector-engine same-engine RAW ordering

    with nc.Block() as block:

        @block.sync
        def _(sync):
            sync.dma_start(idx_sb[0:64, :], idx[0:64, :]).then_inc(io_ixa, 16)
            sync.wait_ge(dn2, 4)
            sync.dma_start(outT[:], otT_sb[0:NT, :]).then_inc(st, 16)
            sync.wait_ge(st, 16)

        @block.scalar
        def _(scalar):
            scalar.dma_start(idx_sb[64:128, :], idx[64:128, :]).then_inc(io_ixb, 16)
            scalar.dma_start(dnt_sb[:], dnt[:]).then_inc(io_dv, 16)
            scalar.dma_start(vdx_sb[:], vdx[:]).then_inc(io_dv, 16)
            # tiles 0-2: h1 = sum(f^2) (square + accumulator), then the final
            # rr = (h2-h1)*0.5 and out_col = rr + linear, all on the
            # otherwise-idle activation engine (h2 comes from the DVE).
            # tile 3: only h1 here; the DVE finishes the tail itself so the
            # serial act chain is never on the critical path.
            for t in range(NT):
                abase = AOPS * t
                scalar.wait_ge(vv, TBASE[t] + 11)
                nc.scalar.activation(
                    out=sqa_sb[:], in_=tot_sb[t][:, :D],
                    func=act_fn.Square, accum_out=h1_sb[t][:],
                ).then_inc(ah if t == NT - 1 else av, 1)
                if t == NT - 1:
                    break
                # rr0 = -0.5 * h1
                scalar.wait_ge(av, abase + 1)
                nc.scalar.activation(
                    out=rr_sb[t][:], in_=h1_sb[t][:],
                    func=act_fn.Copy, scale=-0.5,
                ).then_inc(av, 1)
                # rr = 0.5*h2 + rr0
                scalar.wait_ge(vv, TBASE[t] + 13)
                scalar.wait_ge(av, abase + 2)
                nc.scalar.activation(
                    out=rr_sb[t][:], in_=h2_sb[t][:],
                    func=act_fn.Identity, scale=0.5, bias=rr_sb[t][:],
                ).then_inc(av, 1)
                # out_col = rr + linear
                scalar.wait_ge(av, abase + 3)
                nc.scalar.activation(
                    out=ot4_sb[:, t:t + 1], in_=rr_sb[t][:],
                    func=act_fn.Identity, scale=1.0,
                    bias=tot_sb[t][:, D:DW],
                ).then_inc(ah, 1)
            scalar.wait_ge(st, 16)

        @block.gpsimd
        def _(gpsimd):
            from concourse import library_config as lc

            # the ~9us async Q7 library fetch starts here; regs + waits hide
            # underneath it
            gpsimd.load_library(lc.mlp)
            sizes = sorted({nf for tc in TCHUNKS for _, nf in tc})
            regs = {nf: gpsimd.to_reg(nf * P) for nf in sizes}
            gpsimd.wait_ge(io_ixa, 16)
            gpsimd.wait_ge(io_ixb, 16)
            for t in range(NT):
                g3 = g_sb[t].rearrange("p (c e) -> p c e", e=E)
                for c, (c0, nf) in enumerate(TCHUNKS[t]):
                    gpsimd.dma_gather(
                        g3[:, c0:c0 + nf, :],
                        table[:],
                        idx_sb[:, t * TC + 8 * c0:t * TC + 8 * (c0 + nf)],
                        nf * P,
                        regs[nf],
                        E,
                        single_packet=False,
                        queue_num=TQUEUES[t][c],
                    ).then_inc(gs[t][c], 16)

        @block.tensor
        def _(tensor):
            tensor.wait_ge(io_dv, 32)
            for t in range(NT):
                nc.tensor.matmul(
                    out=ps_ps[t][:],
                    lhsT=dnt_sb[:, t * P:(t + 1) * P],
                    rhs=vdx_sb[:],
                    start=True,
                    stop=True,
                ).then_inc(mm, 1)

        @block.vector
        def _(vector):
            W = DW       # 313 payload columns
            W2 = DW + 1  # even width so fp16 adds take the 2x read-packed mode
            nc.vector.memset(ot4_sb[:], 0.0)
            # vv counts completed vector ops (OPS per tile) so dependent
            # same-engine ops can wait out the deep DVE pipeline
            for t in range(NT):
                base = TBASE[t]
                s = t % 2
                g3 = g_sb[t].rearrange("p (c e) -> p c e", e=E)

                def tt(out, in0, in1, op=add_op):
                    nc.vector.tensor_tensor(out=out, in0=in0, in1=in1, op=op
                                            ).then_inc(vv, 1)

                # op0: blocks[0:7] += blocks[7:14]   (f0-6 += f7-13)
                for c in range(4):  # chunks 0-3 cover f0-13
                    vector.wait_ge(gs[t][c], 16)
                tt(g3[:, 0:7, 0:W2], g3[:, 0:7, 0:W2], g3[:, 7:14, 0:W2])
                # op1: blocks[0:7] += blocks[14:21]  (+= f14-20)
                vector.wait_ge(gs[t][4], 16)
                vector.wait_ge(gs[t][5], 16)
                vector.wait_ge(vv, base + 1)
                tt(g3[:, 0:7, 0:W2], g3[:, 0:7, 0:W2], g3[:, 14:21, 0:W2])
                # op2: blocks[0:3] += blocks[3:6]    [dep op1]
                vector.wait_ge(vv, base + 2)
                tt(g3[:, 0:3, 0:W2], g3[:, 0:3, 0:W2], g3[:, 3:6, 0:W2])
                # op3: b0 += b1                      [dep op2]
                vector.wait_ge(vv, base + 3)
                tt(g3[:, 0, 0:W2], g3[:, 0, 0:W2], g3[:, 1, 0:W2])
                # op4: b2 += b6                      [dep op2]
                vector.wait_ge(vv, base + 3)
                tt(g3[:, 2, 0:W2], g3[:, 2, 0:W2], g3[:, 6, 0:W2])
                # op5: b0 += b2                      [dep op3, op4]
                vector.wait_ge(vv, base + 5)
                tt(g3[:, 0, 0:W2], g3[:, 0, 0:W2], g3[:, 2, 0:W2])
                # op6: b0 += b21 (f21, from chunk C) [dep op5]
                vector.wait_ge(vv, base + 6)
                tt(g3[:, 0, 0:W2], g3[:, 0, 0:W2], g3[:, 21, 0:W2])
                # op7: blocks[22:24] += blocks[24:26]  (D chunk: f22-25)
                vector.wait_ge(gs[t][6], 16)
                tt(g3[:, 22:24, 0:W2], g3[:, 22:24, 0:W2], g3[:, 24:26, 0:W2])
                # op8: b22 += b23                    [dep op7]
                vector.wait_ge(vv, base + 8)
                tt(g3[:, 22, 0:W2], g3[:, 22, 0:W2], g3[:, 23, 0:W2])
                # op9: tot = b0 + b22 (fp32)         [dep op6, op8]
                vector.wait_ge(vv, base + 9)
                tt(tot_sb[t][:], g3[:, 0, 0:W], g3[:, 22, 0:W])
                # op10: tot += dense matmul part     [dep op9 + mm]
                # (act engine fires h1 = sum(tot^2) off vv >= base+11)
                vector.wait_ge(mm, t + 1)
                vector.wait_ge(vv, base + 10)
                tt(tot_sb[t][:], tot_sb[t][:], ps_ps[t][:])
                # op11: s8_k = sum_i f_ik            [dep op10]
                tv = tot_sb[t][:, :D].rearrange("p (i k) -> p k i", k=K)
                vector.wait_ge(vv, base + 11)
                nc.vector.reduce_sum(
                    out=s8_sb[t][:], in_=tv, axis=mybir.AxisListType.X
                ).then_inc(vv, 1)
                # op12: sq8 = s*s with fused row-sum h2  [dep op11]
                # (act engine computes rr and out_col from h1, h2 for t0-2)
                vector.wait_ge(vv, base + 12)
                nc.vector.scalar_tensor_tensor(
                    out=sq8_sb[s][:], in0=s8_sb[t][:], scalar=0.0,
                    in1=s8_sb[t][:], op0=mybir.AluOpType.bypass,
                    op1=mult_op, accum_out=h2_sb[t][:],
                ).then_inc(vv, 1)
                if t == NT - 1:
                    # op13/op14 on DVE for the last tile (act h1 via ah)
                    vector.wait_ge(ah, NT)
                    vector.wait_ge(vv, base + 13)
                    nc.vector.tensor_scalar(
                        out=rr_sb[t][:], in0=h2_sb[t][:], scalar1=h1_sb[t][:],
                        scalar2=0.5, op0=mybir.AluOpType.subtract,
                        op1=mult_op,
                    ).then_inc(vv, 1)
                    vector.wait_ge(vv, base + 14)
                    nc.vector.tensor_scalar(
                        out=ot4_sb[:, t:t + 1], in0=rr_sb[t][:],
                        scalar1=tot_sb[t][:, D:DW], scalar2=None, op0=add_op,
                    ).then_inc(vv, 1)
            # transpose [128, 4used] -> [4used, 128] via 4 32x32 blocks
            # (vv >= 54 implies t3's out col written, which required ah >= 4,
            # which implies the act engine wrote cols 0-2 already)
            vector.wait_ge(vv, TBASE[NT - 1] + TOPS[NT - 1])
            for b in range(4):
                nc.vector.transpose(
                    out=otT_sb[0:32, 32 * b:32 * (b + 1)],
                    in_=ot4_sb[32 * b:32 * (b + 1), 0:32],
                ).then_inc(dn2, 1)

    nc.compile()
    _cached_nc = nc
    return nc


def _prepare_inputs(inputs, w0, w, v):
    dense = np.ascontiguousarray(inputs[:, :N_DENSE].astype(np.float32))
    idx = inputs[:, N_DENSE:].astype(np.int32)
    flat_idx = (N_DENSE + np.arange(N_SPARSE, dtype=np.int32) * ONEHOT)[None, :] + idx

    table = np.zeros((FEAT, E), np.float16)
    table[:, :D] = v.reshape(FEAT, D).astype(np.float16)
    table[:, D] = np.asarray(w, np.float32).reshape(FEAT).astype(np.float16)
    w0_row = np.zeros((1, DW), np.float32)
    w0_row[0, D] = np.asarray(w0, np.float32).reshape(-1)[0]
    vdx_top = np.concatenate(
        [v.reshape(FEAT, D)[:N_DENSE], np.asarray(w, np.float32).reshape(FEAT, 1)[:N_DENSE]],
        axis=1,
    ).astype(np.float32)
    vdx = np.ascontiguousarray(np.concatenate([vdx_top, w0_row], axis=0))

    in_maps = []
    for c in range(NCORES):
        sl = slice(c * BC, (c + 1) * BC)
        dnt = np.concatenate(
            [dense[sl].T, np.ones((1, BC), np.float32)], axis=0
        )  # [14, 512]
        # per tile t the gathers consume indices i = f_local*128 + p, laid
        # out int16 at [i % 16, i // 16] in the first 16 partitions,
        # replicated 8x down the partitions (one copy per Q7 core)
        fi = flat_idx[sl].astype(np.int16)  # [512, 26]
        blocks = []
        for t in range(NT):
            lin = fi[t * P:(t + 1) * P, :].T.reshape(NI)  # field-major
            blk = lin.reshape(TC, 16).T  # [16, TC]
            blocks.append(np.tile(blk, (8, 1)))  # [128, TC]
        idx_buf = np.ascontiguousarray(np.concatenate(blocks, axis=1))
        in_maps.append(
            {
                "table": table,
                "idx": idx_buf,
                "dnt": np.ascontiguousarray(dnt),
                "vdx": vdx,
            }
        )
    return in_maps


def kernel(**inputs):
    from concourse import bass_utils

    nc = _build_program()
    in_maps = _prepare_inputs(
        np.asarray(inputs["inputs"]),
        np.asarray(inputs["w0"]),
        np.asarray(inputs["w"]),
        np.asarray(inputs["v"]),
    )
    res = bass_utils.run_bass_kernel_spmd(nc, in_maps, core_ids=list(range(NCORES)))
    outs = [np.asarray(res.results[c]["out"]) for c in range(NCORES)]
    return np.concatenate(outs, axis=0).astype(np.float32)


# revision 18
# speedup vs baseline: 1.1182x; 1.1182x over previous
"""FFM layer kernel for Trainium2, data-parallel over batch on 8 NeuronCores.

The reference computes, for each sample b:
    x = [dense(13) | onehot(26 fields x 1000)]            # [B, 26013]
    linear = w0 + x @ w                                   # [B, 1]
    field_f = einsum('bf,fik->bik', x, v)                 # [B, 39, 8]
    inter = 0.5*((sum_i field_f)^2.sum(k) - (field_f^2).sum(i,k))
    out = linear + inter

Because x is one-hot in the sparse block, x @ [v|w] is a 26-row gather from
an fp16 [26013, 384] table (cols 0..311 = flattened v row, col 312 = w,
313.. zero pad so each row is 768 B, a multiple of 256) plus a tiny fp32
dense [14]x[14,313] matmul (row 13 = ones row carrying w0 into col 312).
Each core handles 512 samples as 4 tiles of 128.  Each tile's 26
rows/sample are fetched by dma_gather calls; tile 0 uses FIVE chunks
(1/6/7/8/4 fields, the 1-field first chunk unblocks the gpsimd engine's
4-deep exec queue in ~1 us instead of ~7.5 so all four SWDGE queues
start generating descriptors immediately), tiles 1-3 use FOUR chunks
(7/7/8/4).  The queue assignment is hand-balanced so each of the 4
queues carries exactly 3328 rows.  The num_idxs count registers are
loaded once up front and the mlp library load is issued as the first
gpsimd instruction (the ~9 us Q7 library fetch is the hard lower bound
on the first gather).  The idx table is split-loaded on the two HWDGE
queues (sync + scalar).  The 26-row sum runs on the vector engine as a
minimal 10-add tree working in-place in the gather buffers, arranged so
everything except the last 4-field chunk is collapsed before that chunk
lands.  h1 = sum(f^2) plus the final (h2-h1)*0.5 + linear combine run on
the otherwise-idle activation engine in parallel with the DVE's s8
reduction; the [512,1] result is transposed with 4 DVE 32x32 stream
transposes so the final store is 4 contiguous 512 B rows.  Raw bacc
with manual semaphores -- no TileContext.
"""

import numpy as np

N_DENSE = 13
N_SPARSE = 26
ONEHOT = 1000
FIELD = 39
K = 8
FEAT = N_DENSE + N_SPARSE * ONEHOT  # 26013
B = 4096
NCORES = 8
BC = B // NCORES  # 512 samples per core
P = 128
NT = BC // P  # 4 tiles per core
D = FIELD * K  # 312
DW = D + 1  # 313 (col 312 carries the linear weight)
E = 384  # gathered fp16 row width (768 B, multiple of 256)
NI = N_SPARSE * P  # 3328 gathered rows per tile
TC = NI // 16  # 208 idx columns per tile
# per-tile sub-gathers: (first field, num fields).  Tile 0 uses 8 small
# chunks: the 1-field first chunk unblocks the gpsimd engine's exec queue in
# ~1.5 us, and the chunks the first DVE add needs (f0-13) are spread over
# all four SWDGE queues so they land in parallel at the per-queue desc-gen
# rate (~91 GB/s each).  Tiles 1-3 use 4 big chunks rotated across queues:
# small chunks there would re-drain the engine's 4-deep exec queue every
# round and put gaps in the stream.
TCHUNKS = (
    ((0, 1), (1, 3), (4, 3), (7, 4), (11, 3), (14, 4), (18, 4), (22, 4)),
    ((0, 7), (7, 7), (14, 8), (22, 4)),
    ((0, 7), (7, 7), (14, 8), (22, 4)),
    ((0, 7), (7, 7), (14, 8), (22, 4)),
)
TQUEUES = (
    (0, 1, 2, 3, 0, 1, 2, 3),
    (1, 2, 3, 0),
    (2, 3, 0, 1),
    (3, 0, 1, 2),
)
# chunk indices each tree stage waits for: (op0: f0-13, op1: f14-21, op7: f22-25)
TDEPS = (
    ((0, 1, 2, 3, 4), (5, 6), (7,)),
    ((0, 1), (2,), (3,)),
    ((0, 1), (2,), (3,)),
    ((0, 1), (2,), (3,)),
)
# vv increments per tile: tiles 0-2 leave rr/out to the act engine (13 ops);
# tile 3 keeps them on the DVE (15 ops) so the act chain is off the tail
TOPS = (13, 13, 13, 15)
TBASE = (0, 13, 26, 39)
AOPS = 3  # act-engine av increments per tile 0-2 (last op incs ah instead)

_cached_nc = None


def _build_program():
    global _cached_nc
    if _cached_nc is not None:
        return _cached_nc

    import concourse.bacc as bacc
    import concourse.mybir as mybir
    from concourse import library_config

    nc = bacc.Bacc(
        "TRN2",
        debug=False,
        enable_asserts=False,
        target_bir_lowering=False,
        num_devices=NCORES,
        num_swdge_queues=4,
        dynamic_dma_scratch_size=65536,
    )
    f32 = mybir.dt.float32
    f16 = mybir.dt.float16
    i16 = mybir.dt.int16
    add_op = mybir.AluOpType.add
    mult_op = mybir.AluOpType.mult
    act_fn = mybir.ActivationFunctionType

    table = nc.dram_tensor("table", [FEAT, E], f16, kind="ExternalInput").ap()
    idx = nc.dram_tensor("idx", [P, NT * TC], i16, kind="ExternalInput").ap()
    dnt = nc.dram_tensor("dnt", [N_DENSE + 1, BC], f32, kind="ExternalInput").ap()
    vdx = nc.dram_tensor("vdx", [N_DENSE + 1, DW], f32, kind="ExternalInput").ap()
    out = nc.dram_tensor("out", [BC, 1], f32, kind="ExternalOutput").ap()
    outT = out.rearrange("(t p) o -> t (p o)", t=NT)

    idx_sb = nc.alloc_sbuf_tensor("idx_sb", [P, NT * TC], i16).ap()
    dnt_sb = nc.alloc_sbuf_tensor("dnt_sb", [N_DENSE + 1, BC], f32).ap()
    vdx_sb = nc.alloc_sbuf_tensor("vdx_sb", [N_DENSE + 1, DW], f32).ap()
    g_sb = [nc.alloc_sbuf_tensor(f"g{t}", [P, 26 * E], f16).ap() for t in range(NT)]
    tot_sb = [nc.alloc_sbuf_tensor(f"tot{t}", [P, DW], f32).ap() for t in range(NT)]
    s8_sb = [nc.alloc_sbuf_tensor(f"s8_{t}", [P, K], f32).ap() for t in range(NT)]
    sqa_sb = nc.alloc_sbuf_tensor("sqa", [P, D], f16).ap()  # act-engine dump
    sq8_sb = [nc.alloc_sbuf_tensor(f"sq8_{s}", [P, K], f32).ap() for s in range(2)]
    h1_sb = [nc.alloc_sbuf_tensor(f"h1_{t}", [P, 1], f32).ap() for t in range(NT)]
    h2_sb = [nc.alloc_sbuf_tensor(f"h2_{t}", [P, 1], f32).ap() for t in range(NT)]
    rr_sb = [nc.alloc_sbuf_tensor(f"rr_{t}", [P, 1], f32).ap() for t in range(NT)]
    # [128, 32] fp32: col t holds tile t's result; cols 4..31 zeroed once so
    # the 32x32 stream transposes never read uninitialized SBUF
    ot4_sb = nc.alloc_sbuf_tensor("ot4", [P, 32], f32).ap()
    otT_sb = nc.alloc_sbuf_tensor("otT", [32, P], f32).ap()
    ps_ps = [nc.alloc_psum_tensor(f"ps{t}", [P, DW], f32).ap() for t in range(NT)]

    io_ixa = nc.alloc_semaphore("io_ixa")  # idx partitions 0..63 (sync q)
    io_ixb = nc.alloc_semaphore("io_ixb")  # idx partitions 64..127 (scalar q)
    io_dv = nc.alloc_semaphore("io_dv")    # dnt+vdx loads x 16 each
    st = nc.alloc_semaphore("st")          # output store x 16
    # one sem per sub-gather: a DMA sem may only be updated from one SWDGE
    # queue, and completions of two gathers on one queue interleave
    gs = [
        [nc.alloc_semaphore(f"gs{t}_{c}") for c in range(len(TCHUNKS[t]))]
        for t in range(NT)
    ]
    mm = nc.alloc_semaphore("mm")      # dense matmul done (per tile)
    ah = nc.alloc_semaphore("ah")      # act-engine tile output done
    av = nc.alloc_semaphore("av")      # act-engine same-engine RAW ordering
    dn2 = nc.alloc_semaphore("dn2")    # transposed result ready for store
    vv = nc.alloc_semaphore("vv")      # vector-engine same-engine RAW ordering

    with nc.Block() as block:

        @block.sync
        def _(sync):
            sync.dma_start(idx_sb[0:64, :], idx[0:64, :]).then_inc(io_ixa, 16)
            sync.wait_ge(dn2, 4)
            sync.dma_start(outT[:], otT_sb[0:NT, :]).then_inc(st, 16)
            sync.wait_ge(st, 16)

        @block.scalar
        def _(scalar):
            scalar.dma_start(idx_sb[64:128, :], idx[64:128, :]).then_inc(io_ixb, 16)
            scalar.dma_start(dnt_sb[:], dnt[:]).then_inc(io_dv, 16)
            scalar.dma_start(vdx_sb[:], vdx[:]).then_inc(io_dv, 16)
            # tiles 0-2: h1 = sum(f^2) (square + accumulator), then the final
            # rr = (h2-h1)*0.5 and out_col = rr + linear, all on the
            # otherwise-idle activation engine (h2 comes from the DVE).
            # tile 3: only h1 here; the DVE finishes the tail itself so the
            # serial act chain is never on the critical path.
            for t in range(NT):
                abase = AOPS * t
                scalar.wait_ge(vv, TBASE[t] + 11)
                nc.scalar.activation(
                    out=sqa_sb[:], in_=tot_sb[t][:, :D],
                    func=act_fn.Square, accum_out=h1_sb[t][:],
                ).then_inc(ah if t == NT - 1 else av, 1)
                if t == NT - 1:
                    break
                # rr0 = -0.5 * h1
                scalar.wait_ge(av, abase + 1)
                nc.scalar.activation(
                    out=rr_sb[t][:], in_=h1_sb[t][:],
                    func=act_fn.Copy, scale=-0.5,
                ).then_inc(av, 1)
                # rr = 0.5*h2 + rr0
                scalar.wait_ge(vv, TBASE[t] + 13)
                scalar.wait_ge(av, abase + 2)
                nc.scalar.activation(
                    out=rr_sb[t][:], in_=h2_sb[t][:],
                    func=act_fn.Identity, scale=0.5, bias=rr_sb[t][:],
                ).then_inc(av, 1)
                # out_col = rr + linear
                scalar.wait_ge(av, abase + 3)
                nc.scalar.activation(
                    out=ot4_sb[:, t:t + 1], in_=rr_sb[t][:],
                    func=act_fn.Identity, scale=1.0,
                    bias=tot_sb[t][:, D:DW],
                ).then_inc(ah, 1)
            scalar.wait_ge(st, 16)

        @block.gpsimd
        def _(gpsimd):
            from concourse import library_config as lc

            # the ~9us async Q7 library fetch starts here; regs + waits hide
            # underneath it
            gpsimd.load_library(lc.mlp)
            sizes = sorted({nf for tc in TCHUNKS for _, nf in tc})
            regs = {nf: gpsimd.to_reg(nf * P) for nf in sizes}
            gpsimd.wait_ge(io_ixa, 16)
            gpsimd.wait_ge(io_ixb, 16)
            for t in range(NT):
                g3 = g_sb[t].rearrange("p (c e) -> p c e", e=E)
                for c, (c0, nf) in enumerate(TCHUNKS[t]):
                    gpsimd.dma_gather(
                        g3[:, c0:c0 + nf, :],
                        table[:],
                        idx_sb[:, t * TC + 8 * c0:t * TC + 8 * (c0 + nf)],
                        nf * P,
                        regs[nf],
                        E,
                        single_packet=False,
                        queue_num=TQUEUES[t][c],
                    ).then_inc(gs[t][c], 16)

        @block.tensor
        def _(tensor):
            tensor.wait_ge(io_dv, 32)
            for t in range(NT):
                nc.tensor.matmul(
                    out=ps_ps[t][:],
                    lhsT=dnt_sb[:, t * P:(t + 1) * P],
                    rhs=vdx_sb[:],
                    start=True,
                    stop=True,
                ).then_inc(mm, 1)

        @block.vector
        def _(vector):
            W = DW       # 313 payload columns
            W2 = DW + 1  # even width so fp16 adds take the 2x read-packed mode
            nc.vector.memset(ot4_sb[:], 0.0)
            # vv counts completed vector ops (OPS per tile) so dependent
            # same-engine ops can wait out the deep DVE pipeline
            for t in range(NT):
                base = TBASE[t]
                s = t % 2
                g3 = g_sb[t].rearrange("p (c e) -> p c e", e=E)

                def tt(out, in0, in1, op=add_op):
                    nc.vector.tensor_tensor(out=out, in0=in0, in1=in1, op=op
                                            ).then_inc(vv, 1)

                # op0: blocks[0:7] += blocks[7:14]   (f0-6 += f7-13)
                for c in TDEPS[t][0]:
                    vector.wait_ge(gs[t][c], 16)
                tt(g3[:, 0:7, 0:W2], g3[:, 0:7, 0:W2], g3[:, 7:14, 0:W2])
                # op1: blocks[0:7] += blocks[14:21]  (+= f14-20)
                for c in TDEPS[t][1]:
                    vector.wait_ge(gs[t][c], 16)
                vector.wait_ge(vv, base + 1)
                tt(g3[:, 0:7, 0:W2], g3[:, 0:7, 0:W2], g3[:, 14:21, 0:W2])
                # op2: blocks[0:3] += blocks[3:6]    [dep op1]
                vector.wait_ge(vv, base + 2)
                tt(g3[:, 0:3, 0:W2], g3[:, 0:3, 0:W2], g3[:, 3:6, 0:W2])
                # op3: b0 += b1                      [dep op2]
                vector.wait_ge(vv, base + 3)
                tt(g3[:, 0, 0:W2], g3[:, 0, 0:W2], g3[:, 1, 0:W2])
                # op4: b2 += b6                      [dep op2]
                vector.wait_ge(vv, base + 3)
                tt(g3[:, 2, 0:W2], g3[:, 2, 0:W2], g3[:, 6, 0:W2])
                # op5: b0 += b2                      [dep op3, op4]
                vector.wait_ge(vv, base + 5)
                tt(g3[:, 0, 0:W2], g3[:, 0, 0:W2], g3[:, 2, 0:W2])
                # op6: b0 += b21 (f21, from chunk C) [dep op5]
                vector.wait_ge(vv, base + 6)
                tt(g3[:, 0, 0:W2], g3[:, 0, 0:W2], g3[:, 21, 0:W2])
                # op7: blocks[22:24] += blocks[24:26]  (D chunk: f22-25)
                for c in TDEPS[t][2]:
                    vector.wait_ge(gs[t][c], 16)
                tt(g3[:, 22:24, 0:W2], g3[:, 22:24, 0:W2], g3[:, 24:26, 0:W2])
                # op8: b22 += b23                    [dep op7]
                vector.wait_ge(vv, base + 8)
                tt(g3[:, 22, 0:W2], g3[:, 22, 0:W2], g3[:, 23, 0:W2])
                # op9: tot = b0 + b22 (fp32)         [dep op6, op8]
                vector.wait_ge(vv, base + 9)
                tt(tot_sb[t][:], g3[:, 0, 0:W], g3[:, 22, 0:W])
                # op10: tot += dense matmul part     [dep op9 + mm]
                # (act engine fires h1 = sum(tot^2) off vv >= base+11)
                vector.wait_ge(mm, t + 1)
                vector.wait_ge(vv, base + 10)
                tt(tot_sb[t][:], tot_sb[t][:], ps_ps[t][:])
                # op11: s8_k = sum_i f_ik            [dep op10]
                tv = tot_sb[t][:, :D].rearrange("p (i k) -> p k i", k=K)
                vector.wait_ge(vv, base + 11)
                nc.vector.reduce_sum(
                    out=s8_sb[t][:], in_=tv, axis=mybir.AxisListType.X
                ).then_inc(vv, 1)
                # op12: sq8 = s*s with fused row-sum h2  [dep op11]
                # (act engine computes rr and out_col from h1, h2 for t0-2)
                vector.wait_ge(vv, base + 12)
                nc.vector.scalar_tensor_tensor(
                    out=sq8_sb[s][:], in0=s8_sb[t][:], scalar=0.0,
                    in1=s8_sb[t][:], op0=mybir.AluOpType.bypass,
                    op1=mult_op, accum_out=h2_sb[t][:],
                ).then_inc(vv, 1)
                if t == NT - 1:
                    # op13/op14 on DVE for the last tile (act h1 via ah)
                    vector.wait_ge(ah, NT)
                    vector.wait_ge(vv, base + 13)
                    nc.vector.tensor_scalar(
                        out=rr_sb[t][:], in0=h2_sb[t][:], scalar1=h1_sb[t][:],
                        scalar2=0.5, op0=mybir.AluOpType.subtract,
                        op1=mult_op,
                    ).then_inc(vv, 1)
                    vector.wait_ge(vv, base + 14)
                    nc.vector.tensor_scalar(
                        out=ot4_sb[:, t:t + 1], in0=rr_sb[t][:],
                        scalar1=tot_sb[t][:, D:DW], scalar2=None, op0=add_op,
                    ).then_inc(vv, 1)
            # transpose [128, 4used] -> [4used, 128] via 4 32x32 blocks
            # (vv >= 54 implies t3's out col written, which required ah >= 4,
            # which implies the act engine wrote cols 0-2 already)
            vector.wait_ge(vv, TBASE[NT - 1] + TOPS[NT - 1])
            for b in range(4):
                nc.vector.transpose(
                    out=otT_sb[0:32, 32 * b:32 * (b + 1)],
                    in_=ot4_sb[32 * b:32 * (b + 1), 0:32],
                ).then_inc(dn2, 1)

    nc.compile()
    _cached_nc = nc
    return nc


def _prepare_inputs(inputs, w0, w, v):
    dense = np.ascontiguousarray(inputs[:, :N_DENSE].astype(np.float32))
    idx = inputs[:, N_DENSE:].astype(np.int32)
    flat_idx = (N_DENSE + np.arange(N_SPARSE, dtype=np.int32) * ONEHOT)[None, :] + idx

    table = np.zeros((FEAT, E), np.float16)
    table[:, :D] = v.reshape(FEAT, D).astype(np.float16)
    table[:, D] = np.asarray(w, np.float32).reshape(FEAT).astype(np.float16)
    w0_row = np.zeros((1, DW), np.float32)
    w0_row[0, D] = np.asarray(w0, np.float32).reshape(-1)[0]
    vdx_top = np.concatenate(
        [v.reshape(FEAT, D)[:N_DENSE], np.asarray(w, np.float32).reshape(FEAT, 1)[:N_DENSE]],
        axis=1,
    ).astype(np.float32)
    vdx = np.ascontiguousarray(np.concatenate([vdx_top, w0_row], axis=0))

    in_maps = []
    for c in range(NCORES):
        sl = slice(c * BC, (c + 1) * BC)
        dnt = np.concatenate(
            [dense[sl].T, np.ones((1, BC), np.float32)], axis=0
        )  # [14, 512]
        # per tile t the gathers consume indices i = f_local*128 + p, laid
        # out int16 at [i % 16, i // 16] in the first 16 partitions,
        # replicated 8x down the partitions (one copy per Q7 core)
        fi = flat_idx[sl].astype(np.int16)  # [512, 26]
        blocks = []
        for t in range(NT):
            lin = fi[t * P:(t + 1) * P, :].T.reshape(NI)  # field-major
            blk = lin.reshape(TC, 16).T  # [16, TC]
            blocks.append(np.tile(blk, (8, 1)))  # [128, TC]
        idx_buf = np.ascontiguousarray(np.concatenate(blocks, axis=1))
        in_maps.append(
            {
                "table": table,
                "idx": idx_buf,
                "dnt": np.ascontiguousarray(dnt),
                "vdx": vdx,
            }
        )
    return in_maps


def kernel(**inputs):
    from concourse import bass_utils

    nc = _build_program()
    in_maps = _prepare_inputs(
        np.asarray(inputs["inputs"]),
        np.asarray(inputs["w0"]),
        np.asarray(inputs["w"]),
        np.asarray(inputs["v"]),
    )
    res = bass_utils.run_bass_kernel_spmd(nc, in_maps, core_ids=list(range(NCORES)))
    outs = [np.asarray(res.results[c]["out"]) for c in range(NCORES)]
    return np.concatenate(outs, axis=0).astype(np.float32)


# revision 22
# speedup vs baseline: 1.1391x; 1.0187x over previous
"""FFM layer kernel for Trainium2, data-parallel over batch on 8 NeuronCores.

The reference computes, for each sample b:
    x = [dense(13) | onehot(26 fields x 1000)]            # [B, 26013]
    linear = w0 + x @ w                                   # [B, 1]
    field_f = einsum('bf,fik->bik', x, v)                 # [B, 39, 8]
    inter = 0.5*((sum_i field_f)^2.sum(k) - (field_f^2).sum(i,k))
    out = linear + inter

Because x is one-hot in the sparse block, x @ [v|w] is a 26-row gather from
an fp16 [26013, 384] table (cols 0..311 = flattened v row, col 312 = w,
313.. zero pad so each row is 768 B, a multiple of 256) plus a tiny fp32
dense [14]x[14,313] matmul (row 13 = ones row carrying w0 into col 312).
Each core handles 512 samples as 4 tiles of 128.  Each tile's 26
rows/sample are fetched by dma_gather calls; tile 0 uses FIVE chunks
(1/6/7/8/4 fields, the 1-field first chunk unblocks the gpsimd engine's
4-deep exec queue in ~1 us instead of ~7.5 so all four SWDGE queues
start generating descriptors immediately), tiles 1-3 use FOUR chunks
(7/7/8/4).  The queue assignment is hand-balanced so each of the 4
queues carries exactly 3328 rows.  The num_idxs count registers are
loaded once up front and the mlp library load is issued as the first
gpsimd instruction (the ~9 us Q7 library fetch is the hard lower bound
on the first gather).  The idx table is split-loaded on the two HWDGE
queues (sync + scalar).  The 26-row sum runs on the vector engine as a
minimal 10-add tree working in-place in the gather buffers, arranged so
everything except the last 4-field chunk is collapsed before that chunk
lands.  h1 = sum(f^2) plus the final (h2-h1)*0.5 + linear combine run on
the otherwise-idle activation engine in parallel with the DVE's s8
reduction; the [512,1] result is transposed with 4 DVE 32x32 stream
transposes so the final store is 4 contiguous 512 B rows.  Raw bacc
with manual semaphores -- no TileContext.
"""

import numpy as np

N_DENSE = 13
N_SPARSE = 26
ONEHOT = 1000
FIELD = 39
K = 8
FEAT = N_DENSE + N_SPARSE * ONEHOT  # 26013
B = 4096
NCORES = 8
BC = B // NCORES  # 512 samples per core
P = 128
NT = BC // P  # 4 tiles per core
D = FIELD * K  # 312
DW = D + 1  # 313 (col 312 carries the linear weight)
E = 384  # gathered fp16 row width (768 B, multiple of 256)
NI = N_SPARSE * P  # 3328 gathered rows per tile
TC = NI // 16  # 208 idx columns per tile
# per-tile sub-gathers: (first field, num fields).  Tile 0 uses 8 small
# chunks: the 1-field first chunk unblocks the gpsimd engine's exec queue in
# ~1.5 us, and the chunks the first DVE add needs (f0-13) are spread over
# all four SWDGE queues so they land in parallel at the per-queue desc-gen
# rate (~91 GB/s each).  Tiles 1-3 use 4 big chunks rotated across queues:
# small chunks there would re-drain the engine's 4-deep exec queue every
# round and put gaps in the stream.
TCHUNKS = (
    ((0, 1), (1, 3), (4, 3), (7, 4), (11, 3), (14, 4), (18, 4), (22, 4)),
    ((0, 7), (7, 7), (14, 8), (22, 4)),
    ((0, 7), (7, 7), (14, 8), (22, 4)),
    ((0, 7), (7, 7), (14, 8), (22, 4)),
)
TQUEUES = (
    (0, 1, 2, 3, 0, 1, 2, 3),
    (1, 2, 3, 0),
    (2, 3, 0, 1),
    (3, 0, 1, 2),
)
# chunk indices each tree stage waits for: (op0: f0-13, op1: f14-21, op7: f22-25)
TDEPS = (
    ((0, 1, 2, 3, 4), (5, 6), (7,)),
    ((0, 1), (2,), (3,)),
    ((0, 1), (2,), (3,)),
    ((0, 1), (2,), (3,)),
)
# vv increments per tile: tiles 0-2 leave rr/out to the act engine (13 ops);
# tile 3 keeps them on the DVE (15 ops) so the act chain is off the tail
TOPS = (13, 13, 13, 15)
TBASE = (0, 13, 26, 39)
AOPS = 3  # act-engine av increments per tile 0-2 (last op incs ah instead)

_cached_nc = None


def _build_program():
    global _cached_nc
    if _cached_nc is not None:
        return _cached_nc

    import concourse.bacc as bacc
    import concourse.mybir as mybir
    from concourse import library_config

    nc = bacc.Bacc(
        "TRN2",
        debug=False,
        enable_asserts=False,
        target_bir_lowering=False,
        num_devices=NCORES,
        num_swdge_queues=4,
        dynamic_dma_scratch_size=131072,
    )
    f32 = mybir.dt.float32
    f16 = mybir.dt.float16
    i16 = mybir.dt.int16
    add_op = mybir.AluOpType.add
    mult_op = mybir.AluOpType.mult
    act_fn = mybir.ActivationFunctionType

    table = nc.dram_tensor("table", [FEAT, E], f16, kind="ExternalInput").ap()
    idx = nc.dram_tensor("idx", [P, NT * TC], i16, kind="ExternalInput").ap()
    dnt = nc.dram_tensor("dnt", [N_DENSE + 1, BC], f32, kind="ExternalInput").ap()
    vdx = nc.dram_tensor("vdx", [N_DENSE + 1, DW], f32, kind="ExternalInput").ap()
    out = nc.dram_tensor("out", [BC, 1], f32, kind="ExternalOutput").ap()
    outT = out.rearrange("(t p) o -> t (p o)", t=NT)

    idx_sb = nc.alloc_sbuf_tensor("idx_sb", [P, NT * TC], i16).ap()
    dnt_sb = nc.alloc_sbuf_tensor("dnt_sb", [N_DENSE + 1, BC], f32).ap()
    vdx_sb = nc.alloc_sbuf_tensor("vdx_sb", [N_DENSE + 1, DW], f32).ap()
    # 3 gather buffers; tile 3 reuses tile 0's (the gpsimd issue loop waits
    # for the DVE to finish tile 0 before issuing tile 3's gathers)
    g_bufs = [nc.alloc_sbuf_tensor(f"g{i}", [P, 26 * E], f16).ap() for i in range(3)]
    g_sb = [g_bufs[0], g_bufs[1], g_bufs[2], g_bufs[0]]
    tot_sb = [nc.alloc_sbuf_tensor(f"tot{t}", [P, DW], f32).ap() for t in range(NT)]
    s8_sb = [nc.alloc_sbuf_tensor(f"s8_{t}", [P, K], f32).ap() for t in range(NT)]
    sqa_sb = nc.alloc_sbuf_tensor("sqa", [P, D], f16).ap()  # act-engine dump
    sq8_sb = [nc.alloc_sbuf_tensor(f"sq8_{s}", [P, K], f32).ap() for s in range(2)]
    h1_sb = [nc.alloc_sbuf_tensor(f"h1_{t}", [P, 1], f32).ap() for t in range(NT)]
    h2_sb = [nc.alloc_sbuf_tensor(f"h2_{t}", [P, 1], f32).ap() for t in range(NT)]
    rr_sb = [nc.alloc_sbuf_tensor(f"rr_{t}", [P, 1], f32).ap() for t in range(NT)]
    # [128, 32] fp32: col t holds tile t's result; cols 4..31 zeroed once so
    # the 32x32 stream transposes never read uninitialized SBUF
    ot4_sb = nc.alloc_sbuf_tensor("ot4", [P, 32], f32).ap()
    otT_sb = nc.alloc_sbuf_tensor("otT", [32, P], f32).ap()
    ps_ps = [nc.alloc_psum_tensor(f"ps{t}", [P, DW], f32).ap() for t in range(NT)]

    io_ixa = nc.alloc_semaphore("io_ixa")  # idx partitions 0..63 (sync q)
    io_ixb = nc.alloc_semaphore("io_ixb")  # idx partitions 64..127 (scalar q)
    io_dv = nc.alloc_semaphore("io_dv")    # dnt+vdx loads x 16 each
    st = nc.alloc_semaphore("st")          # output store x 16
    # one sem per sub-gather: a DMA sem may only be updated from one SWDGE
    # queue, and completions of two gathers on one queue interleave
    gs = [
        [nc.alloc_semaphore(f"gs{t}_{c}") for c in range(len(TCHUNKS[t]))]
        for t in range(NT)
    ]
    mm = nc.alloc_semaphore("mm")      # dense matmul done (per tile)
    ah = nc.alloc_semaphore("ah")      # act-engine tile output done
    av = nc.alloc_semaphore("av")      # act-engine same-engine RAW ordering
    dn2 = nc.alloc_semaphore("dn2")    # transposed result ready for store
    vv = nc.alloc_semaphore("vv")      # vector-engine same-engine RAW ordering

    with nc.Block() as block:

        @block.sync
        def _(sync):
            sync.dma_start(idx_sb[0:64, :], idx[0:64, :]).then_inc(io_ixa, 16)
            sync.wait_ge(dn2, 4)
            sync.dma_start(outT[:], otT_sb[0:NT, :]).then_inc(st, 16)
            sync.wait_ge(st, 16)

        @block.scalar
        def _(scalar):
            scalar.dma_start(idx_sb[64:128, :], idx[64:128, :]).then_inc(io_ixb, 16)
            scalar.dma_start(dnt_sb[:], dnt[:]).then_inc(io_dv, 16)
            scalar.dma_start(vdx_sb[:], vdx[:]).then_inc(io_dv, 16)
            # tiles 0-2: h1 = sum(f^2) (square + accumulator), then the final
            # rr = (h2-h1)*0.5 and out_col = rr + linear, all on the
            # otherwise-idle activation engine (h2 comes from the DVE).
            # tile 3: only h1 here; the DVE finishes the tail itself so the
            # serial act chain is never on the critical path.
            for t in range(NT):
                abase = AOPS * t
                scalar.wait_ge(vv, TBASE[t] + 11)
                nc.scalar.activation(
                    out=sqa_sb[:], in_=tot_sb[t][:, :D],
                    func=act_fn.Square, accum_out=h1_sb[t][:],
                ).then_inc(ah if t == NT - 1 else av, 1)
                if t == NT - 1:
                    break
                # rr0 = -0.5 * h1
                scalar.wait_ge(av, abase + 1)
                nc.scalar.activation(
                    out=rr_sb[t][:], in_=h1_sb[t][:],
                    func=act_fn.Copy, scale=-0.5,
                ).then_inc(av, 1)
                # rr = 0.5*h2 + rr0
                scalar.wait_ge(vv, TBASE[t] + 13)
                scalar.wait_ge(av, abase + 2)
                nc.scalar.activation(
                    out=rr_sb[t][:], in_=h2_sb[t][:],
                    func=act_fn.Identity, scale=0.5, bias=rr_sb[t][:],
                ).then_inc(av, 1)
                # out_col = rr + linear
                scalar.wait_ge(av, abase + 3)
                nc.scalar.activation(
                    out=ot4_sb[:, t:t + 1], in_=rr_sb[t][:],
                    func=act_fn.Identity, scale=1.0,
                    bias=tot_sb[t][:, D:DW],
                ).then_inc(ah, 1)
            scalar.wait_ge(st, 16)

        @block.gpsimd
        def _(gpsimd):
            from concourse import library_config as lc

            # the ~9us async Q7 library fetch starts here; regs + waits hide
            # underneath it
            gpsimd.load_library(lc.mlp)
            sizes = sorted({nf for tc in TCHUNKS for _, nf in tc})
            regs = {nf: gpsimd.to_reg(nf * P) for nf in sizes}
            gpsimd.wait_ge(io_ixa, 16)
            gpsimd.wait_ge(io_ixb, 16)
            for t in range(NT):
                if t == NT - 1:
                    # tile 3 reuses tile 0's buffer: wait out tile 0's reads
                    gpsimd.wait_ge(vv, TBASE[0] + TOPS[0])
                g3 = g_sb[t].rearrange("p (c e) -> p c e", e=E)
                for c, (c0, nf) in enumerate(TCHUNKS[t]):
                    gpsimd.dma_gather(
                        g3[:, c0:c0 + nf, :],
                        table[:],
                        idx_sb[:, t * TC + 8 * c0:t * TC + 8 * (c0 + nf)],
                        nf * P,
                        regs[nf],
                        E,
                        single_packet=False,
                        queue_num=TQUEUES[t][c],
                    ).then_inc(gs[t][c], 16)

        @block.tensor
        def _(tensor):
            tensor.wait_ge(io_dv, 32)
            for t in range(NT):
                nc.tensor.matmul(
                    out=ps_ps[t][:],
                    lhsT=dnt_sb[:, t * P:(t + 1) * P],
                    rhs=vdx_sb[:],
                    start=True,
                    stop=True,
                ).then_inc(mm, 1)

        @block.vector
        def _(vector):
            W = DW       # 313 payload columns
            W2 = DW + 1  # even width so fp16 adds take the 2x read-packed mode
            nc.vector.memset(ot4_sb[:], 0.0)
            # vv counts completed vector ops (OPS per tile) so dependent
            # same-engine ops can wait out the deep DVE pipeline
            for t in range(NT):
                base = TBASE[t]
                s = t % 2
                g3 = g_sb[t].rearrange("p (c e) -> p c e", e=E)

                def tt(out, in0, in1, op=add_op):
                    nc.vector.tensor_tensor(out=out, in0=in0, in1=in1, op=op
                                            ).then_inc(vv, 1)

                # op0: blocks[0:7] += blocks[7:14]   (f0-6 += f7-13)
                for c in TDEPS[t][0]:
                    vector.wait_ge(gs[t][c], 16)
                tt(g3[:, 0:7, 0:W2], g3[:, 0:7, 0:W2], g3[:, 7:14, 0:W2])
                # op1: blocks[0:7] += blocks[14:21]  (+= f14-20)
                for c in TDEPS[t][1]:
                    vector.wait_ge(gs[t][c], 16)
                vector.wait_ge(vv, base + 1)
                tt(g3[:, 0:7, 0:W2], g3[:, 0:7, 0:W2], g3[:, 14:21, 0:W2])
                # op2: blocks[0:3] += blocks[3:6]    [dep op1]
                vector.wait_ge(vv, base + 2)
                tt(g3[:, 0:3, 0:W2], g3[:, 0:3, 0:W2], g3[:, 3:6, 0:W2])
                # op3: b0 += b1                      [dep op2]
                vector.wait_ge(vv, base + 3)
                tt(g3[:, 0, 0:W2], g3[:, 0, 0:W2], g3[:, 1, 0:W2])
                # op4: b2 += b6                      [dep op2]
                vector.wait_ge(vv, base + 3)
                tt(g3[:, 2, 0:W2], g3[:, 2, 0:W2], g3[:, 6, 0:W2])
                # op5: b0 += b2                      [dep op3, op4]
                vector.wait_ge(vv, base + 5)
                tt(g3[:, 0, 0:W2], g3[:, 0, 0:W2], g3[:, 2, 0:W2])
                # op6: b0 += b21 (f21, from chunk C) [dep op5]
                vector.wait_ge(vv, base + 6)
                tt(g3[:, 0, 0:W2], g3[:, 0, 0:W2], g3[:, 21, 0:W2])
                # op7: blocks[22:24] += blocks[24:26]  (D chunk: f22-25)
                for c in TDEPS[t][2]:
                    vector.wait_ge(gs[t][c], 16)
                tt(g3[:, 22:24, 0:W2], g3[:, 22:24, 0:W2], g3[:, 24:26, 0:W2])
                # op8: b22 += b23                    [dep op7]
                vector.wait_ge(vv, base + 8)
                tt(g3[:, 22, 0:W2], g3[:, 22, 0:W2], g3[:, 23, 0:W2])
                # op9: tot = b0 + b22 (fp32)         [dep op6, op8]
                vector.wait_ge(vv, base + 9)
                tt(tot_sb[t][:], g3[:, 0, 0:W], g3[:, 22, 0:W])
                # op10: tot += dense matmul part     [dep op9 + mm]
                # (act engine fires h1 = sum(tot^2) off vv >= base+11)
                vector.wait_ge(mm, t + 1)
                vector.wait_ge(vv, base + 10)
                tt(tot_sb[t][:], tot_sb[t][:], ps_ps[t][:])
                # op11: s8_k = sum_i f_ik            [dep op10]
                tv = tot_sb[t][:, :D].rearrange("p (i k) -> p k i", k=K)
                vector.wait_ge(vv, base + 11)
                nc.vector.reduce_sum(
                    out=s8_sb[t][:], in_=tv, axis=mybir.AxisListType.X
                ).then_inc(vv, 1)
                # op12: sq8 = s*s with fused row-sum h2  [dep op11]
                # (act engine computes rr and out_col from h1, h2 for t0-2)
                vector.wait_ge(vv, base + 12)
                nc.vector.scalar_tensor_tensor(
                    out=sq8_sb[s][:], in0=s8_sb[t][:], scalar=0.0,
                    in1=s8_sb[t][:], op0=mybir.AluOpType.bypass,
                    op1=mult_op, accum_out=h2_sb[t][:],
                ).then_inc(vv, 1)
                if t == NT - 1:
                    # op13/op14 on DVE for the last tile (act h1 via ah)
                    vector.wait_ge(ah, NT)
                    vector.wait_ge(vv, base + 13)
                    nc.vector.tensor_scalar(
                        out=rr_sb[t][:], in0=h2_sb[t][:], scalar1=h1_sb[t][:],
                        scalar2=0.5, op0=mybir.AluOpType.subtract,
                        op1=mult_op,
                    ).then_inc(vv, 1)
                    vector.wait_ge(vv, base + 14)
                    nc.vector.tensor_scalar(
                        out=ot4_sb[:, t:t + 1], in0=rr_sb[t][:],
                        scalar1=tot_sb[t][:, D:DW], scalar2=None, op0=add_op,
                    ).then_inc(vv, 1)
            # transpose [128, 4used] -> [4used, 128] via 4 32x32 blocks
            # (vv >= 54 implies t3's out col written, which required ah >= 4,
            # which implies the act engine wrote cols 0-2 already)
            vector.wait_ge(vv, TBASE[NT - 1] + TOPS[NT - 1])
            for b in range(4):
                nc.vector.transpose(
                    out=otT_sb[0:32, 32 * b:32 * (b + 1)],
                    in_=ot4_sb[32 * b:32 * (b + 1), 0:32],
                ).then_inc(dn2, 1)

    nc.compile()
    _cached_nc = nc
    return nc


def _prepare_inputs(inputs, w0, w, v):
    dense = np.ascontiguousarray(inputs[:, :N_DENSE].astype(np.float32))
    idx = inputs[:, N_DENSE:].astype(np.int32)
    flat_idx = (N_DENSE + np.arange(N_SPARSE, dtype=np.int32) * ONEHOT)[None, :] + idx

    table = np.zeros((FEAT, E), np.float16)
    table[:, :D] = v.reshape(FEAT, D).astype(np.float16)
    table[:, D] = np.asarray(w, np.float32).reshape(FEAT).astype(np.float16)
    w0_row = np.zeros((1, DW), np.float32)
    w0_row[0, D] = np.asarray(w0, np.float32).reshape(-1)[0]
    vdx_top = np.concatenate(
        [v.reshape(FEAT, D)[:N_DENSE], np.asarray(w, np.float32).reshape(FEAT, 1)[:N_DENSE]],
        axis=1,
    ).astype(np.float32)
    vdx = np.ascontiguousarray(np.concatenate([vdx_top, w0_row], axis=0))

    in_maps = []
    for c in range(NCORES):
        sl = slice(c * BC, (c + 1) * BC)
        dnt = np.concatenate(
            [dense[sl].T, np.ones((1, BC), np.float32)], axis=0
        )  # [14, 512]
        # per tile t the gathers consume indices i = f_local*128 + p, laid
        # out int16 at [i % 16, i // 16] in the first 16 partitions,
        # replicated 8x down the partitions (one copy per Q7 core)
        fi = flat_idx[sl].astype(np.int16)  # [512, 26]
        blocks = []
        for t in range(NT):
            lin = fi[t * P:(t + 1) * P, :].T.reshape(NI)  # field-major
            blk = lin.reshape(TC, 16).T  # [16, TC]
            blocks.append(np.tile(blk, (8, 1)))  # [128, TC]
        idx_buf = np.ascontiguousarray(np.concatenate(blocks, axis=1))
        in_maps.append(
            {
                "table": table,
                "idx": idx_buf,
                "dnt": np.ascontiguousarray(dnt),
                "vdx": vdx,
            }
        )
    return in_maps


def kernel(**inputs):
    from concourse import bass_utils

    nc = _build_program()
    in_maps = _prepare_inputs(
        np.asarray(inputs["inputs"]),
        np.asarray(inputs["w0"]),
        np.asarray(inputs["w"]),
        np.asarray(inputs["v"]),
    )
    res = bass_utils.run_bass_kernel_spmd(nc, in_maps, core_ids=list(range(NCORES)))
    outs = [np.asarray(res.results[c]["out"]) for c in range(NCORES)]
    return np.concatenate(outs, axis=0).astype(np.float32)
